# revision 29
# baseline (speedup 1.0000x reference)
"""CrossCueFusion Trainium2 kernel (8 NeuronCores, SPMD via bass/Tile).

Sharding: core c owns output rows [32c, 32c+32) of the [1,64,256,512]
output, feature rows [8c, 8c+8) of the 64x128 feature map (= attention
query positions [1024c, 1024c+1024)). Features are computed 1/8 per
core and AllGather'd so every core has full K / V for the global
attention; scores are computed transposed (S^T[j,i], j on partitions)
so softmax denominators come free from a ones-column in the AV matmul.

v3 schedule: the kernel is Act(exp)-bound (~129us of exp per core), so
everything else hides under it. Startup DMA is minimized (bands loaded
straight from HBM, conv2/resid2 use K=32 9-step accumulation instead
of SBUF->SBUF shuffle copies), AllGathers kick off as early as
possible with the residual convs filling the wait, Act runs exp only
(biases/relu moved to DVE), and attention runs ich-major so the first
output half's tail (normalize + resid add + store) overlaps the second
half's attention.
"""

import sys

for p in ("/opt/trn_rl_repo", "/opt/trn_rl_repo/concourse"):
    if p not in sys.path:
        sys.path.insert(0, p)

import contextlib

import ml_dtypes
import numpy as np

import concourse.bass as bass
import concourse.mybir as mybir
import concourse.tile as tile
from concourse import bacc
from concourse.bass_utils import run_bass_kernel_spmd

F32 = mybir.dt.float32
BF16 = mybir.dt.bfloat16
BF = ml_dtypes.bfloat16
EXP = mybir.ActivationFunctionType.Exp
ADD = mybir.AluOpType.add
MAX = mybir.AluOpType.max
MULT = mybir.AluOpType.mult

NCORES = 8
H, W = 256, 512
FH, FW = 64, 128  # feature map
HW = FH * FW  # 8192
NJB = HW // 128  # 64 j-blocks
FR = 8  # feature rows per core
ILOC = FR * FW  # 1024 query positions per core
OUTR = 32  # output rows per core
BANDR, BANDW = 37, W + 2  # input band: rows [32c-3, 32c+34), padded width
F1R = 18  # feature conv1 rows per core: abs [16c-1, 16c+17)
R1R = 34  # resid conv1 rows per core: abs [32c-1, 32c+33)
GJB = 3  # score j-blocks per exp group

_CACHE = {}


def _quads(nrows):
    out = []
    for q in range((nrows + 3) // 4):
        out.append((q, min(4, nrows - 4 * q)))
    return out


def _prep(inputs):
    mono = np.asarray(inputs["mono_pseudo_cost"])[0]
    cost = np.asarray(inputs["cost_volume"])[0]
    g = float(np.asarray(inputs["gamma"]).reshape(-1)[0])

    def band(img, c):
        b = np.zeros((32, BANDR, BANDW), np.float32)
        r0 = 32 * c - 3
        lo, hi = max(0, r0), min(H, r0 + BANDR)
        b[:, lo - r0 : hi - r0, 1:513] = img[:, lo:hi, :]
        return b.astype(BF)

    # K=96 weights for m-branch feat conv1 + resid conv1
    w3k = np.zeros((96, 6, 3, 32), np.float32)
    conv_bias = np.zeros((128, 6), np.float32)
    names = [
        ("me_w1", "me_b1"),
        ("xe_w1", "xe_b1"),
        ("me_w2", "me_b2"),
        ("xe_w2", "xe_b2"),
        ("mr_w1", "mr_b1"),
        ("mr_w2", "mr_b2"),
    ]
    for cv, (wn, bn) in enumerate(names):
        w3 = np.asarray(inputs[wn])  # [o, ci, dy, dx]
        for dy in range(3):
            w3k[32 * dy : 32 * dy + 32, cv] = np.transpose(w3[:, :, dy, :], (1, 2, 0))
        conv_bias[:, cv] = np.tile(np.asarray(inputs[bn]), 4)
    w3k = w3k.reshape(96, 6 * 3 * 32).astype(BF)

    # K=32 9-step weights (replicated on all 4 partition bands):
    # cvi 0: xe_w1 (feat1 x), 1: me_w2 (feat2 m), 2: xe_w2 (feat2 x),
    # 3: mr_w2 (resid2)
    w9r = np.zeros((128, 4, 9, 32), np.float32)
    for cvi, wn in enumerate(["xe_w1", "me_w2", "xe_w2", "mr_w2"]):
        w3 = np.asarray(inputs[wn])  # [o, ci, dy, dx]
        for dy in range(3):
            for dx in range(3):
                blk = np.transpose(w3[:, :, dy, dx], (1, 0))  # [ci, o]
                for jj in range(4):
                    w9r[32 * jj : 32 * jj + 32, cvi, 3 * dy + dx] = blk
    w9r = w9r.reshape(128, 4 * 9 * 32).astype(BF)

    # k/q projection weights, col-tripled so the proj matmul writes all
    # 3 partition copies of krep/qrep directly (M=96)
    wkq = np.zeros((32, 384), np.float32)
    bias_kq = np.zeros((96, 4), np.float32)
    for br, (kw, kb, qw, qb) in enumerate(
        [("mk_w", "mk_b", "mq_w", "mq_b"), ("xk_w", "xk_b", "xq_w", "xq_b")]
    ):
        wkq[:, br * 192 : br * 192 + 96] = np.tile(np.asarray(inputs[kw]).T, (1, 3))
        wkq[:, br * 192 + 96 : br * 192 + 192] = np.tile(
            np.asarray(inputs[qw]).T, (1, 3)
        )
        bias_kq[:, br * 2] = np.tile(np.asarray(inputs[kb]), 3)
        bias_kq[:, br * 2 + 1] = np.tile(np.asarray(inputs[qb]), 3)
    wkq = np.tile(wkq, (3, 1)).astype(BF)

    wvT = np.zeros((96, 64), np.float32)
    for br, vw in enumerate(["mv_w", "xv_w"]):
        t = np.asarray(inputs[vw]).T
        for rep in range(3):
            wvT[32 * rep : 32 * rep + 32, br * 32 : br * 32 + 32] = t
    wvT = wvT.astype(BF)

    # stream m output (multi_out) uses multi values (xv); stream x uses mv
    bvg = np.stack(
        [g * np.asarray(inputs["xv_b"]), g * np.asarray(inputs["mv_b"])], axis=1
    ).astype(np.float32)  # [32, 2]

    wxr = np.asarray(inputs["xr_w"]).T.astype(BF)  # [ci, o]
    bias_xr = np.tile(np.asarray(inputs["xr_b"]), 4).reshape(128, 1).astype(np.float32)
    gamma_arr = np.full((128, 1), g, np.float32)

    in_maps = []
    for c in range(NCORES):
        masks = np.zeros((128, 14), np.float32)
        for q in range(5):  # feat1 quads
            for j in range(4):
                r = 16 * c - 1 + 4 * q + j
                masks[32 * j : 32 * j + 32, q] = 1.0 if 0 <= r < 128 else 0.0
        for q in range(9):  # resid1 quads
            for j in range(4):
                r = 32 * c - 1 + 4 * q + j
                masks[32 * j : 32 * j + 32, 5 + q] = 1.0 if 0 <= r < H else 0.0
        in_maps.append(
            {
                "band_m": band(mono, c),
                "band_x": band(cost, c),
                "w3k": w3k,
                "w9r": w9r,
                "conv_bias": conv_bias,
                "masks": masks,
                "wkq": wkq,
                "bias_kq": bias_kq,
                "wvT": wvT,
                "bvg": bvg,
                "wxr": wxr,
                "bias_xr": bias_xr,
                "gamma_in": gamma_arr,
            }
        )
    return in_maps


def build():
    nc = bacc.Bacc(None)
    band_m = nc.declare_dram_parameter("band_m", [32, BANDR, BANDW], BF16, False)
    band_x = nc.declare_dram_parameter("band_x", [32, BANDR, BANDW], BF16, False)
    w3k_d = nc.declare_dram_parameter("w3k", [96, 576], BF16, False)
    w9r_d = nc.declare_dram_parameter("w9r", [128, 1152], BF16, False)
    conv_bias_d = nc.declare_dram_parameter("conv_bias", [128, 6], F32, False)
    masks_d = nc.declare_dram_parameter("masks", [128, 14], F32, False)
    wkq_d = nc.declare_dram_parameter("wkq", [96, 384], BF16, False)
    bias_kq_d = nc.declare_dram_parameter("bias_kq", [96, 4], F32, False)
    wvT_d = nc.declare_dram_parameter("wvT", [96, 64], BF16, False)
    bvg_d = nc.declare_dram_parameter("bvg", [32, 2], F32, False)
    wxr_d = nc.declare_dram_parameter("wxr", [32, 32], BF16, False)
    bias_xr_d = nc.declare_dram_parameter("bias_xr", [128, 1], F32, False)
    gamma_d = nc.declare_dram_parameter("gamma_in", [128, 1], F32, False)
    out_d = nc.declare_dram_parameter("out", [64, OUTR, W], F32, True)

    with tile.TileContext(nc) as tc:
        _emit(nc, tc, locals())
    nc.finalize()
    return nc


def _emit(nc, tc, d):
    band = {0: d["band_m"], 1: d["band_x"]}
    w3k_d, w9r_d, conv_bias_d, masks_d = (
        d["w3k_d"],
        d["w9r_d"],
        d["conv_bias_d"],
        d["masks_d"],
    )
    wkq_d, bias_kq_d, wvT_d = d["wkq_d"], d["bias_kq_d"], d["wvT_d"]
    bvg_d, wxr_d, bias_xr_d, gamma_d = (
        d["bvg_d"],
        d["wxr_d"],
        d["bias_xr_d"],
        d["gamma_d"],
    )
    out_d = d["out_d"]

    ctx = contextlib.ExitStack()
    with ctx:
        persist = ctx.enter_context(tc.tile_pool(name="persist", bufs=1))
        dram = ctx.enter_context(tc.tile_pool(name="dram", bufs=1, space="DRAM"))
        psum = ctx.enter_context(tc.tile_pool(name="psum", bufs=1, space="PSUM"))
        small = ctx.enter_context(tc.tile_pool(name="small", bufs=2))
        tailp = ctx.enter_context(tc.tile_pool(name="tailp", bufs=1))

        # PSUM budget (8 banks): sp_m [128,1536] + sp_x [128,1536] (rings,
        # 3 banks each) + av0/av1 [128,512] (1 bank each). All conv/proj
        # psum reuses the sp rings via tags.
        SPTAG = ("spm", "spx")

        def sp_tile(i, name):
            return psum.tile([128, 1536], F32, name=name, tag=SPTAG[i % 2])

        # ---- persistent tiles ----
        w3k_sb = persist.tile([96, 576], BF16)
        w9r_sb = persist.tile([128, 1152], BF16)
        conv_bias_sb = persist.tile([128, 6], F32)
        masks_sb = persist.tile([128, 14], F32)
        wkq_sb = persist.tile([96, 384], BF16)
        bias_kq_sb = persist.tile([96, 4], F32)
        wvT_sb = persist.tile([96, 64], BF16)
        bvg_sb = persist.tile([32, 2], F32)
        wxr_sb = persist.tile([32, 32], BF16)
        bias_xr_sb = persist.tile([128, 1], F32)
        gamma_sb = persist.tile([128, 1], F32)
        warm = persist.tile([1, 16], F32, name="warm")
        krep = {
            0: persist.tile([96, HW], BF16, name="krep_m"),
            1: persist.tile([96, HW], BF16, name="krep_x"),
        }
        qrep = {
            0: persist.tile([96, ILOC], BF16, name="qrep_m"),
            1: persist.tile([96, ILOC], BF16, name="qrep_x"),
        }
        vt = {
            0: persist.tile([128, NJB * 33], BF16, name="vt_m"),
            1: persist.tile([128, NJB * 33], BF16, name="vt_x"),
        }
        attnrep = {
            0: persist.tile([128, ILOC], BF16, name="attnrep_m"),
            1: persist.tile([128, ILOC], BF16, name="attnrep_x"),
        }
        resid_sum = {
            0: persist.tile([128, FR, 512], BF16, name="resid_sum_m"),
            1: persist.tile([128, FR, 512], BF16, name="resid_sum_x"),
        }

        # weights first on sync (gates convs), tiny tiles after
        for dst, src in [
            (w3k_sb, w3k_d),
            (w9r_sb, w9r_d),
            (conv_bias_sb, conv_bias_d),
            (masks_sb, masks_d),
            (wkq_sb, wkq_d),
            (bias_kq_sb, bias_kq_d),
            (wvT_sb, wvT_d),
            (bvg_sb, bvg_d),
            (wxr_sb, wxr_d),
            (bias_xr_sb, bias_xr_d),
            (gamma_sb, gamma_d),
        ]:
            nc.sync.dma_start(out=dst[:], in_=src[:])


        nc.vector.memset(warm[:], 0.0)
        nc.scalar.activation(warm[:], warm[:], EXP)

        ag_in = {
            0: dram.tile([32, FR, FW], BF16, name="ag_in_m"),
            1: dram.tile([32, FR, FW], BF16, name="ag_in_x"),
        }
        ag_out = {
            0: dram.tile(
                [NCORES, 32, FR, FW], BF16, addr_space="Shared", name="ag_out_m"
            ),
            1: dram.tile(
                [NCORES, 32, FR, FW], BF16, addr_space="Shared", name="ag_out_x"
            ),
        }
        r2d = dram.tile([2, 2, 512], F32)

        nc.vector.memset(vt[0][:], 1.0)
        nc.vector.memset(vt[1][:], 1.0)

        def finish_branch(br, featloc):
            for j in range(4):
                nc.scalar.dma_start(
                    out=ag_in[br][:, j : FR : 4, :],
                    in_=featloc[32 * j : 32 * j + 32, :, :],
                )
            nc.gpsimd.collective_compute(
                "AllGather",
                mybir.AluOpType.bypass,
                replica_groups=[list(range(NCORES))],
                ins=[ag_in[br][:]],
                outs=[ag_out[br][:]],
            )

        # ---- band loads straight from HBM (no SBUF->SBUF replication) ----
        # 3 dy-shifted copies per branch for K=96 convs, each dy loaded
        # directly from HBM, chunked across the sync/scalar/gpsimd queues
        band_x_sb, free_band_x = tc.tile([32, BANDR, BANDW], BF16, name="band_x_sb")
        for r0, r1, eng in (
            (0, 10, nc.sync),
            (10, 19, nc.gpsimd),
            (19, 28, nc.scalar),
            (28, BANDR, nc.sync),
        ):
            eng.dma_start(out=band_x_sb[:, r0:r1, :], in_=band[1][:, r0:r1, :])
        resid1q, free_resid1q = tc.tile([128, 9, BANDW], BF16, name="resid1q")
        nc.vector.memset(resid1q[:], 0.0)
        shift3_m, free_shift3_m = tc.tile([96, BANDR, BANDW], BF16, name="shift3_m")
        dy_engs = {0: nc.sync, 1: nc.gpsimd, 2: nc.scalar}
        for dy in range(3):
            nr = BANDR - dy
            for r0, r1 in ((0, 13), (13, 25), (25, nr)):
                dy_engs[dy].dma_start(
                    out=shift3_m[32 * dy : 32 * dy + 32, r0:r1, :],
                    in_=band[0][:, r0 + dy : r1 + dy, :],
                )

        def conv_branch(br, s3, spb):
            cv1, cv2 = br, 2 + br
            feat1q, free_feat1q = tc.tile([128, 5, 258], BF16, name=f"feat1q_{br}")
            nc.vector.memset(feat1q[:], 0.0)
            for q, jm in _quads(F1R):
                ps = sp_tile(spb + q, f"f1ps_{br}_{q}")
                if s3 is None:
                    # branch x: K=32 9-step from single-copy band (row-tile 0)
                    for j in range(jm):
                        k = 4 * q + j
                        step = 0
                        for dy in range(3):
                            for dx in range(3):
                                nc.tensor.matmul(
                                    ps[32 * j : 32 * j + 32, 0:256],
                                    w9r_sb[0:32, (3 * dy + dx) * 32 : (3 * dy + dx) * 32 + 32],
                                    band_x_sb[:, 2 * k + dy, dx : dx + 511 : 2],
                                    start=(step == 0),
                                    stop=(step == 8),
                                    tile_position=(0, 32 * j),
                                    skip_group_check=True,
                                )
                                step += 1
                else:
                    for dx in range(3):
                        for j in range(jm):
                            nc.tensor.matmul(
                                ps[32 * j : 32 * j + 32, 0:256],
                                w3k_sb[:, (cv1 * 3 + dx) * 32 : (cv1 * 3 + dx) * 32 + 32],
                                s3[:, 2 * (4 * q + j), dx : dx + 511 : 2],
                                start=(dx == 0),
                                stop=(dx == 2),
                                tile_position=(0, 32 * j),
                                skip_group_check=True,
                            )
                pm = 32 * jm
                ev = small.tile([128, 256], F32, tag="ev")
                nc.vector.tensor_scalar(
                    ev[0:pm, :], ps[0:pm, 0:256],
                    conv_bias_sb[0:pm, cv1 : cv1 + 1], 0.0, ADD, MAX,
                )
                nc.vector.tensor_scalar(
                    feat1q[0:pm, q, 1:257], ev[0:pm, :],
                    masks_sb[0:pm, q : q + 1], None, MULT,
                )

            # conv2 input shuffle [96, 17, 258]
            sf2, free_sf2 = tc.tile([96, 17, 258], BF16, name=f"sf2_{br}")
            for dy in range(3):
                for jj in range(4):
                    qs = [
                        q
                        for q, jmq in _quads(F1R)
                        if jj < jmq and dy <= 4 * q + jj < dy + 17
                    ]
                    if not qs:
                        continue
                    q0, q1 = qs[0], qs[-1] + 1
                    r0 = 4 * q0 + jj - dy
                    r1 = r0 + 4 * (q1 - q0 - 1) + 1
                    (nc.gpsimd, nc.sync, nc.scalar)[(3 * dy + jj) % 3].dma_start(
                        out=sf2[32 * dy : 32 * dy + 32, r0:r1:4, :],
                        in_=feat1q[32 * jj : 32 * jj + 32, q0:q1, :],
                    )

            featloc, free_featloc = tc.tile([128, 2, FW], BF16, name=f"featloc_{br}")
            for q in range(2):
                ps = sp_tile(spb + 5 + q, f"f2ps_{br}_{q}")
                for dx in range(3):
                    for j in range(4):
                        nc.tensor.matmul(
                            ps[32 * j : 32 * j + 32, 0:128],
                            w3k_sb[:, (cv2 * 3 + dx) * 32 : (cv2 * 3 + dx) * 32 + 32],
                            sf2[:, 2 * (4 * q + j), dx : dx + 255 : 2],
                            start=(dx == 0),
                            stop=(dx == 2),
                            tile_position=(0, 32 * j),
                            skip_group_check=True,
                        )
                nc.vector.tensor_scalar(
                    featloc[:, q, :], ps[:, 0:128],
                    conv_bias_sb[:, cv2 : cv2 + 1], 0.0, ADD, MAX,
                )
            finish_branch(br, featloc)
            free_featloc()
            free_sf2()
            free_feat1q()

        conv_branch(1, None, 0)
        conv_branch(0, shift3_m, 1)

        # ================= residual convs (fill the AllGather wait) ==========
        for q, jm in _quads(R1R):
            ps = sp_tile(q, f"r1ps_{q}")
            for dx in range(3):
                for j in range(jm):
                    nc.tensor.matmul(
                        ps[32 * j : 32 * j + 32, 0:512],
                        w3k_sb[:, (4 * 3 + dx) * 32 : (4 * 3 + dx) * 32 + 32],
                        shift3_m[:, 4 * q + j + 1, dx : dx + 512],
                        start=(dx == 0),
                        stop=(dx == 2),
                        tile_position=(0, 32 * j),
                        skip_group_check=True,
                    )
            pm = 32 * jm
            ev = small.tile([128, 512], F32, tag="ev2")
            nc.vector.tensor_scalar(
                ev[0:pm, :], ps[0:pm, 0:512],
                conv_bias_sb[0:pm, 4:5], 0.0, ADD, MAX,
            )
            nc.vector.tensor_scalar(
                resid1q[0:pm, q, 1:513], ev[0:pm, :],
                masks_sb[0:pm, 5 + q : 6 + q], None, MULT,
            )
        free_shift3_m()

        sr2, free_sr2 = tc.tile([96, 33, BANDW], BF16, name="sr2")
        for dy in range(3):
            for jj in range(4):
                qs = [
                    q
                    for q, jmq in _quads(R1R)
                    if jj < jmq and dy <= 4 * q + jj < dy + 33
                ]
                if not qs:
                    continue
                q0, q1 = qs[0], qs[-1] + 1
                r0 = 4 * q0 + jj - dy
                r1 = r0 + 4 * (q1 - q0 - 1) + 1
                (nc.sync, nc.scalar, nc.gpsimd)[(3 * dy + jj) % 3].dma_start(
                    out=sr2[32 * dy : 32 * dy + 32, r0:r1:4, :],
                    in_=resid1q[32 * jj : 32 * jj + 32, q0:q1, :],
                )

        # resid conv2 + xr deferred into the attention stream: the exp
        # stream is the attention bottleneck, so the tensor engine has
        # slack to absorb these between score/AV groups.
        def resid_unit(q):
            ps = sp_tile(q, f"r2ps_{q}")
            for dx in range(3):
                for j in range(4):
                    nc.tensor.matmul(
                        ps[32 * j : 32 * j + 32, 0:512],
                        w3k_sb[:, (5 * 3 + dx) * 32 : (5 * 3 + dx) * 32 + 32],
                        sr2[:, 4 * q + j, dx : dx + 512],
                        start=(dx == 0),
                        stop=(dx == 2),
                        tile_position=(0, 32 * j),
                        skip_group_check=True,
                    )
            nc.vector.tensor_scalar(
                resid_sum[0][:, q, :], ps[:, 0:512],
                conv_bias_sb[:, 5:6], 0.0, ADD, MAX,
            )

        def xr_unit(q):
            ps2 = sp_tile(q + 1, f"xps_{q}")
            for j in range(4):
                nc.tensor.matmul(
                    ps2[32 * j : 32 * j + 32, 0:512],
                    wxr_sb[:],
                    band_x_sb[:, 4 * q + 3 + j, 1:513],
                    start=True,
                    stop=True,
                    tile_position=(0, 32 * j),
                )
            nc.vector.tensor_scalar(
                resid_sum[1][:, q, :], ps2[:, 0:512],
                bias_xr_sb[:, 0:1], 0.0, ADD, MAX,
            )

        resid_units = [lambda q=q: resid_unit(q) for q in range(FR)]
        resid_units += [lambda q=q: xr_unit(q) for q in range(FR)]

        # ================= projections (after AllGathers land) ===============
        def proj_branch(br, spb):
            # single-copy feature gather; k/q projections write all 3
            # partition copies via M=96. V^T blocks are deferred into the
            # attention stream (needed only one slot ahead of the AV).
            frep, free_frep = tc.tile([32, HW], BF16, name=f"frep_{br}")
            src_ap = bass.AP(
                tensor=ag_out[br].tensor,
                offset=ag_out[br].offset,
                ap=[
                    [FR * FW, 32],
                    [32 * FR * FW, NCORES],
                    [FW, FR],
                    [1, FW],
                ],
            )
            nc.sync.dma_start(out=frep[:], in_=src_ap)

            for rnd in range(6):
                ch0 = rnd * 3
                take = min(3, 16 - ch0)
                ps = sp_tile(spb + rnd, f"kps_{br}_{rnd}")
                for t in range(take):
                    ch = ch0 + t
                    nc.tensor.matmul(
                        ps[0:96, 512 * t : 512 * t + 512],
                        wkq_sb[0:32, br * 192 : br * 192 + 96],
                        frep[:, 512 * ch : 512 * ch + 512],
                        start=True,
                        stop=True,
                        tile_position=(0, 0),
                    )
                for t in range(take):
                    ch = ch0 + t
                    nc.vector.tensor_scalar(
                        krep[br][0:96, 512 * ch : 512 * ch + 512],
                        ps[0:96, 512 * t : 512 * t + 512],
                        bias_kq_sb[:, br * 2 : br * 2 + 1], None, ADD,
                    )

            qrhs = small.tile([32, ILOC], BF16, name="qrhs", tag="qrhs", bufs=2)
            nc.sync.dma_start(out=qrhs[:], in_=ag_in[br][:])
            ps = sp_tile(spb + 6, f"qps_{br}")
            for t in range(2):
                nc.tensor.matmul(
                    ps[0:96, 512 * t : 512 * t + 512],
                    wkq_sb[0:32, br * 192 + 96 : br * 192 + 192],
                    qrhs[:, 512 * t : 512 * t + 512],
                    start=True,
                    stop=True,
                    tile_position=(0, 0),
                )
            nc.vector.tensor_scalar(
                qrep[br][0:96, :], ps[0:96, 0:1024],
                bias_kq_sb[:, br * 2 + 1 : br * 2 + 2], None, ADD,
            )

            free_frep()
            vtv = vt[br][:].rearrange("p (b c) -> p b c", c=33)

            def vt_unit(gi, g0):
                jbs = list(range(g0, min(g0 + GJB, NJB)))
                fr = small.tile([32, 384], BF16, name=f"fr_{br}_{g0}",
                                tag=f"fr{br}", bufs=2)
                for t, jb in enumerate(jbs):
                    (nc.sync, nc.gpsimd)[(gi + t) % 2].dma_start(
                        out=fr[:, 128 * t : 128 * t + 128],
                        in_=bass.AP(
                            tensor=ag_out[br].tensor,
                            offset=ag_out[br].offset
                            + (jb // FR) * 32 * FR * FW
                            + (jb % FR) * FW,
                            ap=[[FR * FW, 32], [1, FW]],
                        ),
                    )
                ps = sp_tile(gi, f"vtps_{br}_{g0}")
                for t, jb in enumerate(jbs):
                    nc.tensor.matmul(
                        ps[:, 512 * t : 512 * t + 32],
                        fr[:, 128 * t : 128 * t + 128],
                        wvT_sb[0:32, br * 32 : br * 32 + 32],
                        start=True,
                        stop=True,
                        tile_position=(0, 0),
                    )
                psv = ps[:].rearrange("p (t n) -> p t n", n=512)
                nc.vector.tensor_copy(
                    vtv[:, jbs[0] : jbs[0] + len(jbs), 0:32], psv[:, 0 : len(jbs), 0:32]
                )

            units = [
                lambda gi=gi, g0=g0: vt_unit(gi, g0)
                for gi, g0 in enumerate(range(0, NJB, GJB))
            ]
            return units

        vt_units_x = proj_branch(1, 0)
        vt_units_m = proj_branch(0, 1)
        vt_units = [u for pair in zip(vt_units_x, vt_units_m) for u in pair]

        # ================= attention (ich-major) =================
        # av[ich]: rows 0-32 stream m (mono scores x multi V), rows 64-96
        # stream x; row 32/96 hold softmax denominators via the ones
        # column in vt.
        av = {
            ich: psum.tile([128, 512], F32, name=f"av{ich}", tag=f"av{ich}")
            for ich in range(2)
        }
        groups = []
        for ich in range(2):
            for g0 in range(0, NJB, GJB):
                groups.append((ich, g0, list(range(g0, min(g0 + GJB, NJB)))))

        def emit_scores(slot):
            ich, g0, jbs = groups[slot]
            ex = {}
            for br in range(2):
                sp = psum.tile(
                    [128, 1536], F32, name=f"sp_{br}_{g0}_{ich}", tag=SPTAG[br]
                )
                for t, jb in enumerate(jbs):
                    nc.tensor.matmul(
                        sp[:, 512 * t : 512 * t + 512],
                        krep[br][32 * t : 32 * t + 32, 128 * jb : 128 * jb + 128],
                        qrep[br][32 * t : 32 * t + 32, 512 * ich : 512 * ich + 512],
                        start=True,
                        stop=True,
                        tile_position=(32 * t, 0),
                    )
                e = small.tile(
                    [128, 1536], BF16, name=f"ex_{br}_{g0}_{ich}", tag=f"exp{br}",
                    bufs=2,
                )
                n = 512 * len(jbs)
                nc.scalar.activation(e[:, 0:n], sp[:, 0:n], EXP)
                ex[br] = e
            return ex

        def emit_av(slot, ex):
            ich, g0, jbs = groups[slot]
            for t, jb in enumerate(jbs):
                nc.tensor.matmul(
                    av[ich][0:33, :],
                    vt[1][:, 33 * jb : 33 * jb + 33],
                    ex[0][:, 512 * t : 512 * t + 512],
                    start=(jb == 0),
                    stop=(jb == NJB - 1),
                    tile_position=(0, 0),
                    skip_group_check=True,
                )
                nc.tensor.matmul(
                    av[ich][64:97, :],
                    vt[0][:, 33 * jb : 33 * jb + 33],
                    ex[1][:, 512 * t : 512 * t + 512],
                    start=(jb == 0),
                    stop=(jb == NJB - 1),
                    tile_position=(0, 64),
                    skip_group_check=True,
                )

        # ---- per-ich tail: normalize, bias, replicate, resid add, store ----
        rcg = tailp.tile([128, 512], F32, name="rcg")
        out_engs = [nc.sync, nc.gpsimd, nc.sync, nc.gpsimd]

        def emit_tail(ich):
            # reciprocal of the denominator rows, gamma folded in
            for si, p in ((0, 32), (1, 96)):
                nc.vector.reciprocal(rcg[p : p + 1, :], av[ich][p : p + 1, :])
                nc.vector.tensor_scalar(
                    rcg[p : p + 1, :], rcg[p : p + 1, :],
                    gamma_sb[p : p + 1, 0:1], None, MULT,
                )
                nc.sync.dma_start(out=r2d[ich, si, :], in_=rcg[p : p + 1, :])
            for si in range(2):
                base = 64 * si
                rb = tailp.tile([32, 512], F32, name=f"rb_{ich}_{si}", tag="rb", bufs=2)
                src = bass.AP(
                    tensor=r2d.tensor,
                    offset=r2d.offset + (2 * ich + si) * 512,
                    ap=[[0, 32], [1, 512]],
                )
                nc.sync.dma_start(out=rb[:], in_=src)
                t1 = tailp.tile([32, 512], F32, name=f"t1_{ich}_{si}", tag="t1", bufs=2)
                nc.vector.tensor_tensor(t1[:], av[ich][base : base + 32, :], rb[:], MULT)
                nc.vector.tensor_scalar(
                    attnrep[si][0:32, 512 * ich : 512 * ich + 512], t1[:],
                    bvg_sb[:, si : si + 1], None, ADD,
                )
                for rep in range(1, 4):
                    nc.gpsimd.dma_start(
                        out=attnrep[si][32 * rep : 32 * rep + 32, 512 * ich : 512 * ich + 512],
                        in_=attnrep[si][0:32, 512 * ich : 512 * ich + 512],
                    )
            # final adds + store for this ich's 2 feature-row chunks / stream
            for si in range(2):
                for c in (2 * ich, 2 * ich + 1):
                    outst = tailp.tile(
                        [128, 2, 512], F32, name=f"outst_{si}_{c}", tag="outst", bufs=2
                    )
                    eng = nc.gpsimd if c % 2 else nc.vector
                    for k in range(2):
                        q = 2 * c + k
                        eng.tensor_tensor(
                            outst[:, k, :].rearrange("p (a b) -> p a b", b=4),
                            resid_sum[si][:, q, :].rearrange("p (a b) -> p a b", b=4),
                            attnrep[si][:, 128 * q : 128 * q + 128, None].to_broadcast(
                                [128, 128, 4]
                            ),
                            ADD,
                        )
                    for j in range(4):
                        r0 = 8 * c + j
                        out_engs[(2 * c + j) % 4].dma_start(
                            out=out_d[32 * si : 32 * si + 32, r0 : r0 + 5 : 4, :],
                            in_=outst[32 * j : 32 * j + 32, :, :],
                        )

        half = len(groups) // 2
        pending = None
        for slot in range(len(groups)):
            ex = emit_scores(slot)
            for _ in range(4):
                if vt_units:
                    vt_units.pop(0)()
            if not vt_units:
                for _ in range(2):
                    if resid_units:
                        resid_units.pop(0)()
            if pending is not None:
                emit_av(*pending)
                if pending[0] == half - 1:
                    emit_tail(0)
            pending = (slot, ex)
        emit_av(*pending)
        emit_tail(1)
        free_sr2()
        free_resid1q()
        free_band_x()


def kernel(**inputs):
    in_maps = _prep(inputs)
    if "nc" not in _CACHE:
        _CACHE["nc"] = build()
    res = run_bass_kernel_spmd(_CACHE["nc"], in_maps, list(range(NCORES)))
    out = np.concatenate([res.results[c]["out"] for c in range(NCORES)], axis=1)
    return out[None].astype(np.float32)


# revision 30
# speedup vs baseline: 1.0703x; 1.0703x over previous
"""CrossCueFusion Trainium2 kernel (8 NeuronCores, SPMD via bass/Tile).

Sharding: core c owns output rows [32c, 32c+32) of the [1,64,256,512]
output, feature rows [8c, 8c+8) of the 64x128 feature map (= attention
query positions [1024c, 1024c+1024)). Features are computed 1/8 per
core and AllGather'd so every core has full K / V for the global
attention; scores are computed transposed (S^T[j,i], j on partitions)
so softmax denominators come free from a ones-column in the AV matmul.

v3 schedule: the kernel is Act(exp)-bound (~129us of exp per core), so
everything else hides under it. Startup DMA is minimized (bands loaded
straight from HBM, conv2/resid2 use K=32 9-step accumulation instead
of SBUF->SBUF shuffle copies), AllGathers kick off as early as
possible with the residual convs filling the wait, Act runs exp only
(biases/relu moved to DVE), and attention runs ich-major so the first
output half's tail (normalize + resid add + store) overlaps the second
half's attention.
"""

import sys

for p in ("/opt/trn_rl_repo", "/opt/trn_rl_repo/concourse"):
    if p not in sys.path:
        sys.path.insert(0, p)

import contextlib

import ml_dtypes
import numpy as np

import concourse.bass as bass
import concourse.mybir as mybir
import concourse.tile as tile
from concourse import bacc
from concourse.bass_utils import run_bass_kernel_spmd

F32 = mybir.dt.float32
BF16 = mybir.dt.bfloat16
BF = ml_dtypes.bfloat16
EXP = mybir.ActivationFunctionType.Exp
ADD = mybir.AluOpType.add
MAX = mybir.AluOpType.max
MULT = mybir.AluOpType.mult

NCORES = 8
H, W = 256, 512
FH, FW = 64, 128  # feature map
HW = FH * FW  # 8192
NJB = HW // 128  # 64 j-blocks
FR = 8  # feature rows per core
ILOC = FR * FW  # 1024 query positions per core
OUTR = 32  # output rows per core
BANDR, BANDW = 37, W + 2  # input band: rows [32c-3, 32c+34), padded width
F1R = 18  # feature conv1 rows per core: abs [16c-1, 16c+17)
R1R = 34  # resid conv1 rows per core: abs [32c-1, 32c+33)
GJB = 3  # score j-blocks per exp group

_CACHE = {}


def _quads(nrows):
    out = []
    for q in range((nrows + 3) // 4):
        out.append((q, min(4, nrows - 4 * q)))
    return out


def _prep(inputs):
    mono = np.asarray(inputs["mono_pseudo_cost"])[0]
    cost = np.asarray(inputs["cost_volume"])[0]
    g = float(np.asarray(inputs["gamma"]).reshape(-1)[0])

    def band(img, c):
        b = np.zeros((32, BANDR, BANDW), np.float32)
        r0 = 32 * c - 3
        lo, hi = max(0, r0), min(H, r0 + BANDR)
        b[:, lo - r0 : hi - r0, 1:513] = img[:, lo:hi, :]
        return b.astype(BF)

    # K=96 weights for m-branch feat conv1 + resid conv1
    w3k = np.zeros((96, 6, 3, 32), np.float32)
    conv_bias = np.zeros((128, 6), np.float32)
    names = [
        ("me_w1", "me_b1"),
        ("xe_w1", "xe_b1"),
        ("me_w2", "me_b2"),
        ("xe_w2", "xe_b2"),
        ("mr_w1", "mr_b1"),
        ("mr_w2", "mr_b2"),
    ]
    for cv, (wn, bn) in enumerate(names):
        w3 = np.asarray(inputs[wn])  # [o, ci, dy, dx]
        for dy in range(3):
            w3k[32 * dy : 32 * dy + 32, cv] = np.transpose(w3[:, :, dy, :], (1, 2, 0))
        conv_bias[:, cv] = np.tile(np.asarray(inputs[bn]), 4)
    w3k = w3k.reshape(96, 6 * 3 * 32).astype(BF)

    # K=32 9-step weights (replicated on all 4 partition bands):
    # cvi 0: xe_w1 (feat1 x), 1: me_w2 (feat2 m), 2: xe_w2 (feat2 x),
    # 3: mr_w2 (resid2)
    w9r = np.zeros((128, 4, 9, 32), np.float32)
    for cvi, wn in enumerate(["xe_w1", "me_w2", "xe_w2", "mr_w2"]):
        w3 = np.asarray(inputs[wn])  # [o, ci, dy, dx]
        for dy in range(3):
            for dx in range(3):
                blk = np.transpose(w3[:, :, dy, dx], (1, 0))  # [ci, o]
                for jj in range(4):
                    w9r[32 * jj : 32 * jj + 32, cvi, 3 * dy + dx] = blk
    w9r = w9r.reshape(128, 4 * 9 * 32).astype(BF)

    # k/q projection weights, col-tripled so the proj matmul writes all
    # 3 partition copies of krep/qrep directly (M=96)
    wkq = np.zeros((32, 384), np.float32)
    bias_kq = np.zeros((96, 4), np.float32)
    for br, (kw, kb, qw, qb) in enumerate(
        [("mk_w", "mk_b", "mq_w", "mq_b"), ("xk_w", "xk_b", "xq_w", "xq_b")]
    ):
        wkq[:, br * 192 : br * 192 + 96] = np.tile(np.asarray(inputs[kw]).T, (1, 3))
        wkq[:, br * 192 + 96 : br * 192 + 192] = np.tile(
            np.asarray(inputs[qw]).T, (1, 3)
        )
        bias_kq[:, br * 2] = np.tile(np.asarray(inputs[kb]), 3)
        bias_kq[:, br * 2 + 1] = np.tile(np.asarray(inputs[qb]), 3)
    wkq = np.tile(wkq, (3, 1)).astype(BF)

    wvT = np.zeros((96, 64), np.float32)
    for br, vw in enumerate(["mv_w", "xv_w"]):
        t = np.asarray(inputs[vw]).T
        for rep in range(3):
            wvT[32 * rep : 32 * rep + 32, br * 32 : br * 32 + 32] = t
    wvT = wvT.astype(BF)

    # stream m output (multi_out) uses multi values (xv); stream x uses mv
    bvg = np.stack(
        [g * np.asarray(inputs["xv_b"]), g * np.asarray(inputs["mv_b"])], axis=1
    ).astype(np.float32)  # [32, 2]

    wxr = np.asarray(inputs["xr_w"]).T.astype(BF)  # [ci, o]
    bias_xr = np.tile(np.asarray(inputs["xr_b"]), 4).reshape(128, 1).astype(np.float32)
    gamma_arr = np.full((128, 1), g, np.float32)

    in_maps = []
    for c in range(NCORES):
        masks = np.zeros((128, 14), np.float32)
        for q in range(5):  # feat1 quads
            for j in range(4):
                r = 16 * c - 1 + 4 * q + j
                masks[32 * j : 32 * j + 32, q] = 1.0 if 0 <= r < 128 else 0.0
        for q in range(9):  # resid1 quads
            for j in range(4):
                r = 32 * c - 1 + 4 * q + j
                masks[32 * j : 32 * j + 32, 5 + q] = 1.0 if 0 <= r < H else 0.0
        in_maps.append(
            {
                "band_m": band(mono, c),
                "band_x": band(cost, c),
                "w3k": w3k,
                "w9r": w9r,
                "conv_bias": conv_bias,
                "masks": masks,
                "wkq": wkq,
                "bias_kq": bias_kq,
                "wvT": wvT,
                "bvg": bvg,
                "wxr": wxr,
                "bias_xr": bias_xr,
                "gamma_in": gamma_arr,
            }
        )
    return in_maps


def build():
    nc = bacc.Bacc(None)
    band_m = nc.declare_dram_parameter("band_m", [32, BANDR, BANDW], BF16, False)
    band_x = nc.declare_dram_parameter("band_x", [32, BANDR, BANDW], BF16, False)
    w3k_d = nc.declare_dram_parameter("w3k", [96, 576], BF16, False)
    w9r_d = nc.declare_dram_parameter("w9r", [128, 1152], BF16, False)
    conv_bias_d = nc.declare_dram_parameter("conv_bias", [128, 6], F32, False)
    masks_d = nc.declare_dram_parameter("masks", [128, 14], F32, False)
    wkq_d = nc.declare_dram_parameter("wkq", [96, 384], BF16, False)
    bias_kq_d = nc.declare_dram_parameter("bias_kq", [96, 4], F32, False)
    wvT_d = nc.declare_dram_parameter("wvT", [96, 64], BF16, False)
    bvg_d = nc.declare_dram_parameter("bvg", [32, 2], F32, False)
    wxr_d = nc.declare_dram_parameter("wxr", [32, 32], BF16, False)
    bias_xr_d = nc.declare_dram_parameter("bias_xr", [128, 1], F32, False)
    gamma_d = nc.declare_dram_parameter("gamma_in", [128, 1], F32, False)
    out_d = nc.declare_dram_parameter("out", [64, OUTR, W], F32, True)

    with tile.TileContext(nc) as tc:
        _emit(nc, tc, locals())
    nc.finalize()
    return nc


def _emit(nc, tc, d):
    band = {0: d["band_m"], 1: d["band_x"]}
    w3k_d, w9r_d, conv_bias_d, masks_d = (
        d["w3k_d"],
        d["w9r_d"],
        d["conv_bias_d"],
        d["masks_d"],
    )
    wkq_d, bias_kq_d, wvT_d = d["wkq_d"], d["bias_kq_d"], d["wvT_d"]
    bvg_d, wxr_d, bias_xr_d, gamma_d = (
        d["bvg_d"],
        d["wxr_d"],
        d["bias_xr_d"],
        d["gamma_d"],
    )
    out_d = d["out_d"]

    ctx = contextlib.ExitStack()
    with ctx:
        persist = ctx.enter_context(tc.tile_pool(name="persist", bufs=1))
        dram = ctx.enter_context(tc.tile_pool(name="dram", bufs=1, space="DRAM"))
        psum = ctx.enter_context(tc.tile_pool(name="psum", bufs=1, space="PSUM"))
        small = ctx.enter_context(tc.tile_pool(name="small", bufs=2))
        tailp = ctx.enter_context(tc.tile_pool(name="tailp", bufs=1))

        # PSUM budget (8 banks): sp_m [128,1536] + sp_x [128,1536] (rings,
        # 3 banks each) + av0/av1 [128,512] (1 bank each). All conv/proj
        # psum reuses the sp rings via tags.
        SPTAG = ("spm", "spx")

        def sp_tile(i, name):
            return psum.tile([128, 1536], F32, name=name, tag=SPTAG[i % 2])

        # ---- persistent tiles ----
        w3k_sb = persist.tile([96, 576], BF16)
        w9r_sb = persist.tile([128, 1152], BF16)
        conv_bias_sb = persist.tile([128, 6], F32)
        masks_sb = persist.tile([128, 14], F32)
        wkq_sb = persist.tile([96, 384], BF16)
        bias_kq_sb = persist.tile([96, 4], F32)
        wvT_sb = persist.tile([96, 64], BF16)
        bvg_sb = persist.tile([32, 2], F32)
        wxr_sb = persist.tile([32, 32], BF16)
        bias_xr_sb = persist.tile([128, 1], F32)
        gamma_sb = persist.tile([128, 1], F32)
        warm = persist.tile([1, 16], F32, name="warm")
        krep = {
            0: persist.tile([96, HW], BF16, name="krep_m"),
            1: persist.tile([96, HW], BF16, name="krep_x"),
        }
        qrep = {
            0: persist.tile([96, ILOC], BF16, name="qrep_m"),
            1: persist.tile([96, ILOC], BF16, name="qrep_x"),
        }
        vt = {
            0: persist.tile([128, NJB * 33], BF16, name="vt_m"),
            1: persist.tile([128, NJB * 33], BF16, name="vt_x"),
        }
        attnrep = {
            0: persist.tile([128, ILOC], BF16, name="attnrep_m"),
            1: persist.tile([128, ILOC], BF16, name="attnrep_x"),
        }
        resid_sum = {
            0: persist.tile([128, FR, 512], BF16, name="resid_sum_m"),
            1: persist.tile([128, FR, 512], BF16, name="resid_sum_x"),
        }

        # weights first on sync (gates convs), tiny tiles after
        for dst, src in [
            (w3k_sb, w3k_d),
            (w9r_sb, w9r_d),
            (conv_bias_sb, conv_bias_d),
            (masks_sb, masks_d),
            (wkq_sb, wkq_d),
            (bias_kq_sb, bias_kq_d),
            (wvT_sb, wvT_d),
            (bvg_sb, bvg_d),
            (wxr_sb, wxr_d),
            (bias_xr_sb, bias_xr_d),
            (gamma_sb, gamma_d),
        ]:
            nc.sync.dma_start(out=dst[:], in_=src[:])


        nc.vector.memset(warm[:], 0.0)
        nc.scalar.activation(warm[:], warm[:], EXP)

        ag_in = {
            0: dram.tile([32, FR, FW], BF16, name="ag_in_m"),
            1: dram.tile([32, FR, FW], BF16, name="ag_in_x"),
        }
        ag_out = {
            0: dram.tile(
                [NCORES, 32, FR, FW], BF16, addr_space="Shared", name="ag_out_m"
            ),
            1: dram.tile(
                [NCORES, 32, FR, FW], BF16, addr_space="Shared", name="ag_out_x"
            ),
        }
        r2d = dram.tile([2, 2, 512], F32)

        nc.vector.memset(vt[0][:], 1.0)
        nc.vector.memset(vt[1][:], 1.0)

        def finish_branch(br, featloc):
            for j in range(4):
                nc.scalar.dma_start(
                    out=ag_in[br][:, j : FR : 4, :],
                    in_=featloc[32 * j : 32 * j + 32, :, :],
                )
            nc.gpsimd.collective_compute(
                "AllGather",
                mybir.AluOpType.bypass,
                replica_groups=[list(range(NCORES))],
                ins=[ag_in[br][:]],
                outs=[ag_out[br][:]],
            )

        # ---- band loads straight from HBM (no SBUF->SBUF replication) ----
        # 3 dy-shifted copies per branch for K=96 convs, each dy loaded
        # directly from HBM, chunked across the sync/scalar/gpsimd queues
        band_x_sb, free_band_x = tc.tile([32, BANDR, BANDW], BF16, name="band_x_sb")
        for r0, r1, eng in (
            (0, 10, nc.sync),
            (10, 19, nc.gpsimd),
            (19, 28, nc.scalar),
            (28, BANDR, nc.sync),
        ):
            eng.dma_start(out=band_x_sb[:, r0:r1, :], in_=band[1][:, r0:r1, :])
        resid1q, free_resid1q = tc.tile([128, 9, BANDW], BF16, name="resid1q")
        nc.vector.memset(resid1q[:], 0.0)
        shift3_m, free_shift3_m = tc.tile([96, BANDR, BANDW], BF16, name="shift3_m")
        dy_engs = {0: nc.sync, 1: nc.gpsimd, 2: nc.scalar}
        for dy in range(3):
            nr = BANDR - dy
            for r0, r1 in ((0, 13), (13, 25), (25, nr)):
                dy_engs[dy].dma_start(
                    out=shift3_m[32 * dy : 32 * dy + 32, r0:r1, :],
                    in_=band[0][:, r0 + dy : r1 + dy, :],
                )

        def conv_branch(br, s3, spb):
            cv1, cv2 = br, 2 + br
            feat1q, free_feat1q = tc.tile([128, 5, 258], BF16, name=f"feat1q_{br}")
            nc.vector.memset(feat1q[:], 0.0)
            for q, jm in _quads(F1R):
                ps = sp_tile(spb + q, f"f1ps_{br}_{q}")
                if s3 is None:
                    # branch x: K=32 9-step from single-copy band (row-tile 0)
                    for j in range(jm):
                        k = 4 * q + j
                        step = 0
                        for dy in range(3):
                            for dx in range(3):
                                nc.tensor.matmul(
                                    ps[32 * j : 32 * j + 32, 0:256],
                                    w9r_sb[0:32, (3 * dy + dx) * 32 : (3 * dy + dx) * 32 + 32],
                                    band_x_sb[:, 2 * k + dy, dx : dx + 511 : 2],
                                    start=(step == 0),
                                    stop=(step == 8),
                                    tile_position=(0, 32 * j),
                                    skip_group_check=True,
                                )
                                step += 1
                else:
                    for dx in range(3):
                        for j in range(jm):
                            nc.tensor.matmul(
                                ps[32 * j : 32 * j + 32, 0:256],
                                w3k_sb[:, (cv1 * 3 + dx) * 32 : (cv1 * 3 + dx) * 32 + 32],
                                s3[:, 2 * (4 * q + j), dx : dx + 511 : 2],
                                start=(dx == 0),
                                stop=(dx == 2),
                                tile_position=(0, 32 * j),
                                skip_group_check=True,
                            )
                pm = 32 * jm
                ev = small.tile([128, 256], F32, tag="ev")
                nc.vector.tensor_scalar(
                    ev[0:pm, :], ps[0:pm, 0:256],
                    conv_bias_sb[0:pm, cv1 : cv1 + 1], 0.0, ADD, MAX,
                )
                nc.vector.tensor_scalar(
                    feat1q[0:pm, q, 1:257], ev[0:pm, :],
                    masks_sb[0:pm, q : q + 1], None, MULT,
                )

            # conv2 input shuffle [96, 17, 258]
            sf2, free_sf2 = tc.tile([96, 17, 258], BF16, name=f"sf2_{br}")
            for dy in range(3):
                for jj in range(4):
                    qs = [
                        q
                        for q, jmq in _quads(F1R)
                        if jj < jmq and dy <= 4 * q + jj < dy + 17
                    ]
                    if not qs:
                        continue
                    q0, q1 = qs[0], qs[-1] + 1
                    r0 = 4 * q0 + jj - dy
                    r1 = r0 + 4 * (q1 - q0 - 1) + 1
                    (nc.gpsimd, nc.sync, nc.scalar)[(3 * dy + jj) % 3].dma_start(
                        out=sf2[32 * dy : 32 * dy + 32, r0:r1:4, :],
                        in_=feat1q[32 * jj : 32 * jj + 32, q0:q1, :],
                    )

            featloc, free_featloc = tc.tile([128, 2, FW], BF16, name=f"featloc_{br}")
            for q in range(2):
                ps = sp_tile(spb + 5 + q, f"f2ps_{br}_{q}")
                for dx in range(3):
                    for j in range(4):
                        nc.tensor.matmul(
                            ps[32 * j : 32 * j + 32, 0:128],
                            w3k_sb[:, (cv2 * 3 + dx) * 32 : (cv2 * 3 + dx) * 32 + 32],
                            sf2[:, 2 * (4 * q + j), dx : dx + 255 : 2],
                            start=(dx == 0),
                            stop=(dx == 2),
                            tile_position=(0, 32 * j),
                            skip_group_check=True,
                        )
                nc.vector.tensor_scalar(
                    featloc[:, q, :], ps[:, 0:128],
                    conv_bias_sb[:, cv2 : cv2 + 1], 0.0, ADD, MAX,
                )
            finish_branch(br, featloc)
            free_featloc()
            free_sf2()
            free_feat1q()

        conv_branch(1, None, 0)
        conv_branch(0, shift3_m, 1)

        # ================= residual convs (fill the AllGather wait) ==========
        for q, jm in _quads(R1R):
            ps = sp_tile(q, f"r1ps_{q}")
            for dx in range(3):
                for j in range(jm):
                    nc.tensor.matmul(
                        ps[32 * j : 32 * j + 32, 0:512],
                        w3k_sb[:, (4 * 3 + dx) * 32 : (4 * 3 + dx) * 32 + 32],
                        shift3_m[:, 4 * q + j + 1, dx : dx + 512],
                        start=(dx == 0),
                        stop=(dx == 2),
                        tile_position=(0, 32 * j),
                        skip_group_check=True,
                    )
            pm = 32 * jm
            ev = small.tile([128, 512], F32, tag="ev2")
            nc.vector.tensor_scalar(
                ev[0:pm, :], ps[0:pm, 0:512],
                conv_bias_sb[0:pm, 4:5], 0.0, ADD, MAX,
            )
            nc.vector.tensor_scalar(
                resid1q[0:pm, q, 1:513], ev[0:pm, :],
                masks_sb[0:pm, 5 + q : 6 + q], None, MULT,
            )
        free_shift3_m()

        sr2, free_sr2 = tc.tile([96, 33, BANDW], BF16, name="sr2")
        for dy in range(3):
            for jj in range(4):
                qs = [
                    q
                    for q, jmq in _quads(R1R)
                    if jj < jmq and dy <= 4 * q + jj < dy + 33
                ]
                if not qs:
                    continue
                q0, q1 = qs[0], qs[-1] + 1
                r0 = 4 * q0 + jj - dy
                r1 = r0 + 4 * (q1 - q0 - 1) + 1
                (nc.sync, nc.scalar, nc.gpsimd)[(3 * dy + jj) % 3].dma_start(
                    out=sr2[32 * dy : 32 * dy + 32, r0:r1:4, :],
                    in_=resid1q[32 * jj : 32 * jj + 32, q0:q1, :],
                )

        # resid conv2 + xr deferred into the attention stream: the exp
        # stream is the attention bottleneck, so the tensor engine has
        # slack to absorb these between score/AV groups.
        def resid_unit(q):
            ps = sp_tile(q, f"r2ps_{q}")
            for dx in range(3):
                for j in range(4):
                    nc.tensor.matmul(
                        ps[32 * j : 32 * j + 32, 0:512],
                        w3k_sb[:, (5 * 3 + dx) * 32 : (5 * 3 + dx) * 32 + 32],
                        sr2[:, 4 * q + j, dx : dx + 512],
                        start=(dx == 0),
                        stop=(dx == 2),
                        tile_position=(0, 32 * j),
                        skip_group_check=True,
                    )
            nc.vector.tensor_scalar(
                resid_sum[0][:, q, :], ps[:, 0:512],
                conv_bias_sb[:, 5:6], 0.0, ADD, MAX,
            )

        def xr_unit(q):
            ps2 = sp_tile(q + 1, f"xps_{q}")
            for j in range(4):
                nc.tensor.matmul(
                    ps2[32 * j : 32 * j + 32, 0:512],
                    wxr_sb[:],
                    band_x_sb[:, 4 * q + 3 + j, 1:513],
                    start=True,
                    stop=True,
                    tile_position=(0, 32 * j),
                )
            nc.vector.tensor_scalar(
                resid_sum[1][:, q, :], ps2[:, 0:512],
                bias_xr_sb[:, 0:1], 0.0, ADD, MAX,
            )

        resid_units = [lambda q=q: resid_unit(q) for q in range(FR)]
        resid_units += [lambda q=q: xr_unit(q) for q in range(FR)]

        # ================= projections (after AllGathers land) ===============
        def proj_branch(br, spb):
            # single-copy feature gather; k/q projections write all 3
            # partition copies via M=96. V^T blocks are deferred into the
            # attention stream (needed only one slot ahead of the AV).
            frep, free_frep = tc.tile([32, HW], BF16, name=f"frep_{br}")
            src_ap = bass.AP(
                tensor=ag_out[br].tensor,
                offset=ag_out[br].offset,
                ap=[
                    [FR * FW, 32],
                    [32 * FR * FW, NCORES],
                    [FW, FR],
                    [1, FW],
                ],
            )
            nc.sync.dma_start(out=frep[:], in_=src_ap)

            for rnd in range(6):
                ch0 = rnd * 3
                take = min(3, 16 - ch0)
                ps = sp_tile(spb + rnd, f"kps_{br}_{rnd}")
                for t in range(take):
                    ch = ch0 + t
                    nc.tensor.matmul(
                        ps[0:96, 512 * t : 512 * t + 512],
                        wkq_sb[0:32, br * 192 : br * 192 + 96],
                        frep[:, 512 * ch : 512 * ch + 512],
                        start=True,
                        stop=True,
                        tile_position=(0, 0),
                    )
                for t in range(take):
                    ch = ch0 + t
                    nc.vector.tensor_scalar(
                        krep[br][0:96, 512 * ch : 512 * ch + 512],
                        ps[0:96, 512 * t : 512 * t + 512],
                        bias_kq_sb[:, br * 2 : br * 2 + 1], None, ADD,
                    )

            qrhs = small.tile([32, ILOC], BF16, name="qrhs", tag="qrhs", bufs=2)
            nc.sync.dma_start(out=qrhs[:], in_=ag_in[br][:])
            ps = sp_tile(spb + 6, f"qps_{br}")
            for t in range(2):
                nc.tensor.matmul(
                    ps[0:96, 512 * t : 512 * t + 512],
                    wkq_sb[0:32, br * 192 + 96 : br * 192 + 192],
                    qrhs[:, 512 * t : 512 * t + 512],
                    start=True,
                    stop=True,
                    tile_position=(0, 0),
                )
            nc.vector.tensor_scalar(
                qrep[br][0:96, :], ps[0:96, 0:1024],
                bias_kq_sb[:, br * 2 + 1 : br * 2 + 2], None, ADD,
            )

            vtv = vt[br][:].rearrange("p (b c) -> p b c", c=33)
            for gi, g0 in enumerate(range(0, NJB, GJB)):
                jbs = list(range(g0, min(g0 + GJB, NJB)))
                ps = sp_tile(gi, f"vtps_{br}_{g0}")
                for t, jb in enumerate(jbs):
                    nc.tensor.matmul(
                        ps[:, 512 * t : 512 * t + 32],
                        frep[:, 128 * jb : 128 * jb + 128],
                        wvT_sb[0:32, br * 32 : br * 32 + 32],
                        start=True,
                        stop=True,
                        tile_position=(0, 0),
                    )
                psv = ps[:].rearrange("p (t n) -> p t n", n=512)
                nc.vector.tensor_copy(
                    vtv[:, jbs[0] : jbs[0] + len(jbs), 0:32], psv[:, 0 : len(jbs), 0:32]
                )
            free_frep()

        proj_branch(1, 0)
        proj_branch(0, 1)

        # ================= attention (ich-major) =================
        # av[ich]: rows 0-32 stream m (mono scores x multi V), rows 64-96
        # stream x; row 32/96 hold softmax denominators via the ones
        # column in vt.
        av = {
            ich: psum.tile([128, 512], F32, name=f"av{ich}", tag=f"av{ich}")
            for ich in range(2)
        }
        groups = []
        for ich in range(2):
            for g0 in range(0, NJB, GJB):
                groups.append((ich, g0, list(range(g0, min(g0 + GJB, NJB)))))

        def emit_scores(slot):
            ich, g0, jbs = groups[slot]
            ex = {}
            for br in range(2):
                sp = psum.tile(
                    [128, 1536], F32, name=f"sp_{br}_{g0}_{ich}", tag=SPTAG[br]
                )
                for t, jb in enumerate(jbs):
                    nc.tensor.matmul(
                        sp[:, 512 * t : 512 * t + 512],
                        krep[br][32 * t : 32 * t + 32, 128 * jb : 128 * jb + 128],
                        qrep[br][32 * t : 32 * t + 32, 512 * ich : 512 * ich + 512],
                        start=True,
                        stop=True,
                        tile_position=(32 * t, 0),
                    )
                e = small.tile(
                    [128, 1536], BF16, name=f"ex_{br}_{g0}_{ich}", tag=f"exp{br}",
                    bufs=2,
                )
                n = 512 * len(jbs)
                nc.scalar.activation(e[:, 0:n], sp[:, 0:n], EXP)
                ex[br] = e
            return ex

        def emit_av(slot, ex):
            ich, g0, jbs = groups[slot]
            for t, jb in enumerate(jbs):
                nc.tensor.matmul(
                    av[ich][0:33, :],
                    vt[1][:, 33 * jb : 33 * jb + 33],
                    ex[0][:, 512 * t : 512 * t + 512],
                    start=(jb == 0),
                    stop=(jb == NJB - 1),
                    tile_position=(0, 0),
                    skip_group_check=True,
                )
                nc.tensor.matmul(
                    av[ich][64:97, :],
                    vt[0][:, 33 * jb : 33 * jb + 33],
                    ex[1][:, 512 * t : 512 * t + 512],
                    start=(jb == 0),
                    stop=(jb == NJB - 1),
                    tile_position=(0, 64),
                    skip_group_check=True,
                )

        # ---- per-ich tail: normalize, bias, replicate, resid add, store ----
        rcg = tailp.tile([128, 512], F32, name="rcg")
        out_engs = [nc.sync, nc.gpsimd, nc.sync, nc.gpsimd]

        def emit_tail(ich):
            # reciprocal of the denominator rows, gamma folded in
            for si, p in ((0, 32), (1, 96)):
                nc.vector.reciprocal(rcg[p : p + 1, :], av[ich][p : p + 1, :])
                nc.vector.tensor_scalar(
                    rcg[p : p + 1, :], rcg[p : p + 1, :],
                    gamma_sb[p : p + 1, 0:1], None, MULT,
                )
                nc.sync.dma_start(out=r2d[ich, si, :], in_=rcg[p : p + 1, :])
            for si in range(2):
                base = 64 * si
                rb = tailp.tile([32, 512], F32, name=f"rb_{ich}_{si}", tag="rb", bufs=2)
                src = bass.AP(
                    tensor=r2d.tensor,
                    offset=r2d.offset + (2 * ich + si) * 512,
                    ap=[[0, 32], [1, 512]],
                )
                nc.sync.dma_start(out=rb[:], in_=src)
                t1 = tailp.tile([32, 512], F32, name=f"t1_{ich}_{si}", tag="t1", bufs=2)
                nc.vector.tensor_tensor(t1[:], av[ich][base : base + 32, :], rb[:], MULT)
                nc.vector.tensor_scalar(
                    attnrep[si][0:32, 512 * ich : 512 * ich + 512], t1[:],
                    bvg_sb[:, si : si + 1], None, ADD,
                )
                for rep in range(1, 4):
                    nc.gpsimd.dma_start(
                        out=attnrep[si][32 * rep : 32 * rep + 32, 512 * ich : 512 * ich + 512],
                        in_=attnrep[si][0:32, 512 * ich : 512 * ich + 512],
                    )
            # final adds + store for this ich's 2 feature-row chunks / stream
            for si in range(2):
                for c in (2 * ich, 2 * ich + 1):
                    outst = tailp.tile(
                        [128, 2, 512], F32, name=f"outst_{si}_{c}", tag="outst", bufs=2
                    )
                    eng = nc.gpsimd if c % 2 else nc.vector
                    for k in range(2):
                        q = 2 * c + k
                        eng.tensor_tensor(
                            outst[:, k, :].rearrange("p (a b) -> p a b", b=4),
                            resid_sum[si][:, q, :].rearrange("p (a b) -> p a b", b=4),
                            attnrep[si][:, 128 * q : 128 * q + 128, None].to_broadcast(
                                [128, 128, 4]
                            ),
                            ADD,
                        )
                    for j in range(4):
                        r0 = 8 * c + j
                        out_engs[(2 * c + j) % 4].dma_start(
                            out=out_d[32 * si : 32 * si + 32, r0 : r0 + 5 : 4, :],
                            in_=outst[32 * j : 32 * j + 32, :, :],
                        )

        half = len(groups) // 2
        pending = None
        for slot in range(len(groups)):
            ex = emit_scores(slot)
            for _ in range(2):
                if resid_units:
                    resid_units.pop(0)()
            if pending is not None:
                emit_av(*pending)
                if pending[0] == half - 1:
                    emit_tail(0)
            pending = (slot, ex)
        emit_av(*pending)
        emit_tail(1)
        free_sr2()
        free_resid1q()
        free_band_x()


def kernel(**inputs):
    in_maps = _prep(inputs)
    if "nc" not in _CACHE:
        _CACHE["nc"] = build()
    res = run_bass_kernel_spmd(_CACHE["nc"], in_maps, list(range(NCORES)))
    out = np.concatenate([res.results[c]["out"] for c in range(NCORES)], axis=1)
    return out[None].astype(np.float32)
